# revision 29
# baseline (speedup 1.0000x reference)
"""Trainium2 Bass kernel for nn_ButterflyRotation (B=8192, D=4096, L=12).

Strategy (pure data parallel over 8 cores, 1024 batch rows each):

The 12 butterfly layers factor as T = T2 . T1 where
  - T1 (layers 0-6, strides 1..64) is block-diagonal over 32 outer blocks:
    a 128x128 rotation A_o acting on the inner index q = j[6:0].
  - T2 (layers 7-11, strides 128..2048) mixes only the outer index
    o = j[11:7] (32 values) with coefficients depending on q: for each q a
    32x32 matrix B_q. Packed 4-per-128-partitions as block-diagonal 128x128
    matrices WB_v over partitions p = j[6:5]*32 + o, one per v = j[4:0].

Both factors are applied with TensorE matmuls in fp32r (full-rate fp32 mode,
~1.2e-4 rounding). The mid-pipeline layout swap j[4:0] <-> j[11:7] is exactly
the DVE 32x32 stream-transpose. Weight matrices are composed on the host from
the angles (tiny O(L*d*128) prep, analogous to RoPE cos/sin tables).

Per-group (256 batch rows) device pipeline:
  DMA in -> PE transpose (batch->d-major, fp32r) -> stage A matmuls
  (fp32r, N=256, 1 cyc/row) -> evac with o-minor scatter (split ACT/DVE)
  -> DVE 32x32 stream transpose -> stage B matmuls (plain fp32, lhsT = the
  data so the output lands batch-on-partitions) -> ACT/DVE evac scattering
  d back to natural order -> DMA out.

Measured ~100 us/core on HW (paired-slope method; sessions range 95-120 us
from axon-tunnel timing noise), at the HBM roofline: 16 MiB in + 16 MiB out
+ 2.56 MiB weights @ ~345 GB/s ~= 100 us. Stage-B weights ship as their
nonzero 32x32 blocks only; input DMA uses 1 KiB per-partition bursts.
End-to-end rel l2 error vs the fp32 reference: ~1.3e-4 (fp32r rounds
mantissas to ~13 bits on the input path; stage B is exact fp32).
"""

from contextlib import ExitStack

import numpy as np

import concourse.bass as bass  # noqa: F401 (kept for clarity)
import concourse.tile as tile
from concourse import bacc, mybir
from concourse import bass_utils

F32 = mybir.dt.float32
F32R = mybir.dt.float32r

DIM = 4096
LAYERS = 12
BATCH = 8192
N_CORES = 8
BC = BATCH // N_CORES          # 1024 batch rows per core
GROUP = 256                    # batch rows per pipeline group
NGRP = BC // GROUP             # 4
NB_O = 32                      # outer blocks j[11:7]
NQ = 128                       # inner j[6:0]

_cache = {}


# ---------------------------------------------------------------- host math
def _apply_layers(x, angles, layers):
    B, d = x.shape
    out = x
    for l in layers:
        stride = 1 << l
        nb = d // (2 * stride)
        theta = angles[l].reshape(nb, stride)
        c = np.cos(theta)
        s = np.sin(theta)
        o = out.reshape(B, nb, 2, stride)
        xl = o[:, :, 0, :]
        xr = o[:, :, 1, :]
        new_l = c * xl + s * xr
        new_r = -s * xl + c * xr
        out = np.stack([new_l, new_r], axis=2).reshape(B, d)
    return out


def _build_weights(angles):
    """WA[o][q,q'] = lhsT for stage A; WB[v][p,p'] = lhsT for stage B."""
    a64 = angles.astype(np.float64)
    I = np.eye(DIM, dtype=np.float64)
    M1 = _apply_layers(I, a64, range(0, 7))     # = T1^T (block diagonal)
    M2 = _apply_layers(I, a64, range(7, 12))    # = T2^T (q-diagonal)

    WA = np.zeros((NB_O, NQ, NQ), dtype=np.float32)
    for o in range(NB_O):
        WA[o] = M1[o*128:(o+1)*128, o*128:(o+1)*128].astype(np.float32)

    # WB is block-diagonal: ship only the nonzero 32x32 blocks.
    # WBblk[j65, v] = B-block for q = j65*32 + v (lhsT orientation).
    WBblk = np.zeros((4, 32, 32, 32), dtype=np.float32)
    for j65 in range(4):
        for v in range(32):
            q = j65 * 32 + v
            WBblk[j65, v] = M2[q::128, q::128].astype(np.float32)
    return WA, WBblk


# ---------------------------------------------------------------- device IR
def _build_program(reps=1):
    nc = bacc.Bacc("TRN2", target_bir_lowering=False, debug=False,
                   num_devices=N_CORES)
    x_d = nc.dram_tensor("x", [BC, DIM], F32R, kind="ExternalInput").ap()
    wa_d = nc.dram_tensor("wa", [NB_O, 128, 128], F32R,
                          kind="ExternalInput").ap()
    wb_d = nc.dram_tensor("wb", [4, 32, 32, 32], F32,
                          kind="ExternalInput").ap()
    id_d = nc.dram_tensor("ident", [128, 128], F32R,
                          kind="ExternalInput").ap()
    out_d = nc.dram_tensor("out", [BC, DIM], F32, kind="ExternalOutput").ap()

    with tile.TileContext(nc, trace_sim=False) as tc, ExitStack() as ctx:
        wpool = ctx.enter_context(tc.tile_pool(name="w", bufs=1))
        xpool = ctx.enter_context(tc.tile_pool(name="xin", bufs=5))
        zpool = ctx.enter_context(tc.tile_pool(name="z", bufs=3))
        opool = ctx.enter_context(tc.tile_pool(name="xout", bufs=2))
        pt_in = ctx.enter_context(tc.tile_pool(name="ptin", bufs=2,
                                               space="PSUM"))
        pa = ctx.enter_context(tc.tile_pool(name="pa", bufs=4, space="PSUM"))
        pb = ctx.enter_context(tc.tile_pool(name="pb", bufs=2, space="PSUM"))

        wa_sb = wpool.tile([128, NB_O * 128], F32R, tag="wa")
        wb_sb = wpool.tile([128, 32 * 128], F32, tag="wb")
        ident = wpool.tile([128, 128], F32R, tag="ident")
        # weights go through the gpsimd software DGE so they don't head-block
        # the x-piece loads on the sync HWDGE queue
        nc.sync.dma_start(ident[:], id_d[:])
        # wa in quarters so stage A's first matmuls only wait on 0.5 MiB
        for k in range(4):
            nc.gpsimd.dma_start(
                wa_sb[:].rearrange("q (o m) -> q o m", m=128)[:, 8*k:8*k+8],
                wa_d[8*k:8*k+8].rearrange("o q m -> q o m"))
        # wb_sb is block-diagonal: zero it once, then land only the 32x32
        # blocks (4 DMAs, one per partition quarter j65)
        nc.gpsimd.memset(wb_sb[:], 0.0)
        for j65 in range(4):
            dst = wb_sb[j65*32:(j65+1)*32, :].rearrange(
                "o (v m) -> o v m", m=128)[:, :, j65*32:(j65+1)*32]
            nc.gpsimd.dma_start(dst, wb_d[j65].rearrange("v o m -> o v m"))

        # HAM warm-up: ~3 us of dummy matmuls during the otherwise-idle DMA
        # head so the PE clock-gate is at 2.4 GHz when real work arrives
        for i in range(28):
            pw = pa.tile([128, 128], F32, tag="pa", name=f"warm_{i}")
            nc.tensor.matmul(pw[:], ident[:], ident[:])

        for g in [g for _ in range(reps) for g in range(NGRP)]:
            # Z slabs for this group (rotating through zpool's slots)
            z1 = zpool.tile([128, NB_O * GROUP], F32R, tag="slab")   # [q,(o,b)]
            z2 = zpool.tile([128, NB_O * GROUP], F32, tag="slab")    # [q,(b,o)]
            z3 = zpool.tile([128, NB_O * GROUP], F32, tag="slab")    # [p,(b,v)]

            # --- phase 1: load + transpose to d-major ---------------------
            for c in range(2):
                row0 = g * GROUP + c * 128
                for p in range(4):          # pieces of 8 o-columns (1KB rows)
                    xp = xpool.tile([128, 1024], F32R, tag="xp")
                    nc.sync.dma_start(xp[:],
                                      x_d[row0:row0 + 128,
                                          p * 1024:(p + 1) * 1024])
                    for h in range(2):      # 4 transposes per psum tile
                        pt = pt_in.tile([128, 512], F32R, tag="pt")
                        for oo in range(4):
                            nc.tensor.transpose(
                                pt[:, oo*128:(oo+1)*128],
                                xp[:, h*512 + oo*128:h*512 + (oo+1)*128],
                                ident[:])
                        # evac: Z1 free = o*GROUP + c*128 + b (split ACT/DVE)
                        o0 = 8 * p + 4 * h
                        dst = z1[:].rearrange("q (o b) -> q o b", b=GROUP)[
                            :, o0:o0 + 4, c*128:(c+1)*128]
                        src = pt[:].rearrange("q (o b) -> q o b", b=128)
                        if (2 * p + h) % 4 == 3:
                            nc.vector.tensor_copy(dst, src)
                        else:
                            nc.scalar.copy(dst, src)

            # --- phase 2: stage A matmuls --------------------------------
            for o in range(NB_O):
                ps_a = pa.tile([128, GROUP], F32, tag="pa")
                nc.tensor.matmul(ps_a[:],
                                 wa_sb[:, o*128:(o+1)*128],
                                 z1[:, o*GROUP:(o+1)*GROUP])
                # evac scatter: Z2 free = b*32 + o  (split DVE/ACT)
                dst = z2[:].rearrange("q (b o) -> q b o", o=32)[:, :, o]
                if o % 2 == 0:
                    nc.vector.tensor_copy(dst, ps_a[:])
                else:
                    nc.scalar.copy(dst, ps_a[:])

            # --- phase 3: 32x32 stream transpose -------------------------
            for s in range(16):
                sl = slice(s * 512, (s + 1) * 512)
                nc.vector.transpose(z3[:, sl], z2[:, sl])

            # --- phase 4: stage B matmuls (lhsT = data) ------------------
            z3v = z3[:].rearrange("p (b v) -> p b v", v=32)
            xo_tiles = [opool.tile([128, DIM], F32, tag="xo", name=f"xo_{g}_{c}")
                        for c in range(2)]
            for c in range(2):
                for vq in range(8):
                    ps_b = pb.tile([128, 512], F32, tag="pb")
                    for vv in range(4):
                        v = vq * 4 + vv
                        lhsT = z3v[:, c*128:(c+1)*128, v]    # [p, b] strided
                        nc.tensor.matmul(ps_b[:, vv*128:(vv+1)*128],
                                         lhsT,
                                         wb_sb[:, v*128:(v+1)*128])
                    # evac scatter: out free j' = o'*128 + j65*32 + v
                    dst = xo_tiles[c][:].rearrange(
                        "b (o f v) -> b v f o", f=4, v=32)[
                        :, vq*4:(vq+1)*4, :, :]
                    src = ps_b[:].rearrange("b (v f o) -> b v f o", v=4, f=4)
                    if vq % 4 == 3:
                        nc.vector.tensor_copy(dst, src)
                    else:
                        nc.scalar.copy(dst, src)

            # --- phase 5: store ------------------------------------------
            for c in range(2):
                row0 = g * GROUP + c * 128
                nc.sync.dma_start(out_d[row0:row0 + 128, :], xo_tiles[c][:])

    nc.compile()
    return nc


def _get_program():
    if "nc" not in _cache:
        _cache["nc"] = _build_program()
    return _cache["nc"]


# ---------------------------------------------------------------- entry
def kernel(x, angles):
    x = np.ascontiguousarray(np.asarray(x, dtype=np.float32))
    angles = np.asarray(angles, dtype=np.float32)
    assert x.shape == (BATCH, DIM) and angles.shape == (LAYERS, DIM // 2)

    WA, WB = _build_weights(angles)
    ident = np.eye(128, dtype=np.float32)
    nc = _get_program()

    in_maps = []
    for core in range(N_CORES):
        in_maps.append({
            "x": np.ascontiguousarray(x[core * BC:(core + 1) * BC]),
            "wa": WA, "wb": WB, "ident": ident,
        })
    res = bass_utils.run_bass_kernel_spmd(
        nc, in_maps, core_ids=list(range(N_CORES)))
    out = np.concatenate([r["out"] for r in res.results], axis=0)
    return out


# revision 31
# speedup vs baseline: 1.4119x; 1.4119x over previous
"""Trainium2 Bass kernel for nn_ButterflyRotation (B=8192, D=4096, L=12).

Strategy (pure data parallel over 8 cores, 1024 batch rows each):

The 12 butterfly layers factor as T = T2 . T1 where
  - T1 (layers 0-6, strides 1..64) is block-diagonal over 32 outer blocks:
    a 128x128 rotation A_o acting on the inner index q = j[6:0].
  - T2 (layers 7-11, strides 128..2048) mixes only the outer index
    o = j[11:7] (32 values) with coefficients depending on q: for each q a
    32x32 matrix B_q. Packed 4-per-128-partitions as block-diagonal 128x128
    matrices WB_v over partitions p = j[6:5]*32 + o, one per v = j[4:0].

Both factors are applied with TensorE matmuls in fp32r (full-rate fp32 mode,
~1.2e-4 rounding). The mid-pipeline layout swap j[4:0] <-> j[11:7] is exactly
the DVE 32x32 stream-transpose. Weight matrices are composed on the host from
the angles (tiny O(L*d*128) prep, analogous to RoPE cos/sin tables).

Per-group (256 batch rows) device pipeline:
  DMA in -> PE transpose (batch->d-major, fp32r) -> stage A matmuls
  (fp32r, N=256, 1 cyc/row) -> evac with o-minor scatter (split ACT/DVE)
  -> DVE 32x32 stream transpose -> stage B matmuls (plain fp32, lhsT = the
  data so the output lands batch-on-partitions) -> ACT/DVE evac scattering
  d back to natural order -> DMA out.

Measured ~100 us/core on HW (paired-slope method; sessions range 95-120 us
from axon-tunnel timing noise), at the HBM roofline: 16 MiB in + 16 MiB out
+ 2.56 MiB weights @ ~345 GB/s ~= 100 us. Stage-B weights ship as their
nonzero 32x32 blocks only; input DMA uses 1 KiB per-partition bursts.
End-to-end rel l2 error vs the fp32 reference: ~1.3e-4 (fp32r rounds
mantissas to ~13 bits on the input path; stage B is exact fp32).
"""

from contextlib import ExitStack

import numpy as np

import concourse.bass as bass  # noqa: F401 (kept for clarity)
import concourse.tile as tile
from concourse import bacc, mybir
from concourse import bass_utils

F32 = mybir.dt.float32
F32R = mybir.dt.float32r

DIM = 4096
LAYERS = 12
BATCH = 8192
N_CORES = 8
BC = BATCH // N_CORES          # 1024 batch rows per core
GROUP = 256                    # batch rows per pipeline group
NGRP = BC // GROUP             # 4
NB_O = 32                      # outer blocks j[11:7]
NQ = 128                       # inner j[6:0]

_cache = {}


# ---------------------------------------------------------------- host math
def _apply_layers(x, angles, layers):
    B, d = x.shape
    out = x
    for l in layers:
        stride = 1 << l
        nb = d // (2 * stride)
        theta = angles[l].reshape(nb, stride)
        c = np.cos(theta)
        s = np.sin(theta)
        o = out.reshape(B, nb, 2, stride)
        xl = o[:, :, 0, :]
        xr = o[:, :, 1, :]
        new_l = c * xl + s * xr
        new_r = -s * xl + c * xr
        out = np.stack([new_l, new_r], axis=2).reshape(B, d)
    return out


def _build_weights(angles):
    """WA[o][q,q'] = lhsT for stage A; WB[v][p,p'] = lhsT for stage B."""
    a64 = angles.astype(np.float64)
    I = np.eye(DIM, dtype=np.float64)
    M1 = _apply_layers(I, a64, range(0, 7))     # = T1^T (block diagonal)
    M2 = _apply_layers(I, a64, range(7, 12))    # = T2^T (q-diagonal)

    WA = np.zeros((NB_O, NQ, NQ), dtype=np.float32)
    for o in range(NB_O):
        WA[o] = M1[o*128:(o+1)*128, o*128:(o+1)*128].astype(np.float32)

    # WB is block-diagonal: ship only the nonzero 32x32 blocks.
    # WBblk[j65, v] = B-block for q = j65*32 + v (lhsT orientation).
    WBblk = np.zeros((4, 32, 32, 32), dtype=np.float32)
    for j65 in range(4):
        for v in range(32):
            q = j65 * 32 + v
            WBblk[j65, v] = M2[q::128, q::128].astype(np.float32)
    return WA, WBblk


# ---------------------------------------------------------------- device IR
def _build_program(reps=1):
    nc = bacc.Bacc("TRN2", target_bir_lowering=False, debug=False,
                   num_devices=N_CORES)
    x_d = nc.dram_tensor("x", [BC, DIM], F32R, kind="ExternalInput").ap()
    wa_d = nc.dram_tensor("wa", [NB_O, 128, 128], F32R,
                          kind="ExternalInput").ap()
    wb_d = nc.dram_tensor("wb", [4, 32, 32, 32], F32,
                          kind="ExternalInput").ap()
    id_d = nc.dram_tensor("ident", [128, 128], F32R,
                          kind="ExternalInput").ap()
    out_d = nc.dram_tensor("out", [BC, DIM], F32, kind="ExternalOutput").ap()

    with tile.TileContext(nc, trace_sim=False) as tc, ExitStack() as ctx:
        wpool = ctx.enter_context(tc.tile_pool(name="w", bufs=1))
        xpool = ctx.enter_context(tc.tile_pool(name="xin", bufs=5))
        zpool = ctx.enter_context(tc.tile_pool(name="z", bufs=3))
        opool = ctx.enter_context(tc.tile_pool(name="xout", bufs=2))
        pt_in = ctx.enter_context(tc.tile_pool(name="ptin", bufs=2,
                                               space="PSUM"))
        pa = ctx.enter_context(tc.tile_pool(name="pa", bufs=4, space="PSUM"))
        pb = ctx.enter_context(tc.tile_pool(name="pb", bufs=2, space="PSUM"))

        wa_sb = wpool.tile([128, NB_O * 128], F32R, tag="wa")
        wb_sb = wpool.tile([128, 32 * 128], F32, tag="wb")
        ident = wpool.tile([128, 128], F32R, tag="ident")
        # weights go through the gpsimd software DGE so they don't head-block
        # the x-piece loads on the sync HWDGE queue
        nc.sync.dma_start(ident[:], id_d[:])
        # wa in quarters so stage A's first matmuls only wait on 0.5 MiB
        for k in range(4):
            nc.gpsimd.dma_start(
                wa_sb[:].rearrange("q (o m) -> q o m", m=128)[:, 8*k:8*k+8],
                wa_d[8*k:8*k+8].rearrange("o q m -> q o m"))
        # wb_sb is block-diagonal: zero it once, then land only the 32x32
        # blocks (4 DMAs, one per partition quarter j65)
        nc.gpsimd.memset(wb_sb[:], 0.0)
        for j65 in range(4):
            dst = wb_sb[j65*32:(j65+1)*32, :].rearrange(
                "o (v m) -> o v m", m=128)[:, :, j65*32:(j65+1)*32]
            nc.gpsimd.dma_start(dst, wb_d[j65].rearrange("v o m -> o v m"))

        # HAM warm-up: ~3 us of dummy matmuls during the otherwise-idle DMA
        # head so the PE clock-gate is at 2.4 GHz when real work arrives
        for i in range(28):
            pw = pa.tile([128, 128], F32, tag="pa", name=f"warm_{i}")
            nc.tensor.matmul(pw[:], ident[:], ident[:])

        for g in [g for _ in range(reps) for g in range(NGRP)]:
            # Z slabs for this group (rotating through zpool's slots)
            z1 = zpool.tile([128, NB_O * GROUP], F32R, tag="slab")   # [q,(o,b)]
            z2 = zpool.tile([128, NB_O * GROUP], F32, tag="slab")    # [q,(b,o)]
            z3 = zpool.tile([128, NB_O * GROUP], F32, tag="slab")    # [p,(b,v)]

            # --- phase 1: load + transpose to d-major ---------------------
            for c in range(2):
                row0 = g * GROUP + c * 128
                for p in range(4):          # pieces of 8 o-columns (1KB rows)
                    xp = xpool.tile([128, 1024], F32R, tag="xp")
                    nc.sync.dma_start(xp[:],
                                      x_d[row0:row0 + 128,
                                          p * 1024:(p + 1) * 1024])
                    for h in range(2):      # 4 transposes per psum tile
                        pt = pt_in.tile([128, 512], F32R, tag="pt")
                        for oo in range(4):
                            nc.tensor.transpose(
                                pt[:, oo*128:(oo+1)*128],
                                xp[:, h*512 + oo*128:h*512 + (oo+1)*128],
                                ident[:])
                        # evac: Z1 free = o*GROUP + c*128 + b (split ACT/DVE)
                        o0 = 8 * p + 4 * h
                        dst = z1[:].rearrange("q (o b) -> q o b", b=GROUP)[
                            :, o0:o0 + 4, c*128:(c+1)*128]
                        src = pt[:].rearrange("q (o b) -> q o b", b=128)
                        if (2 * p + h) % 4 == 3:
                            nc.vector.tensor_copy(dst, src)
                        else:
                            nc.scalar.copy(dst, src)

            # --- phase 2: stage A matmuls --------------------------------
            for o in range(NB_O):
                ps_a = pa.tile([128, GROUP], F32, tag="pa")
                nc.tensor.matmul(ps_a[:],
                                 wa_sb[:, o*128:(o+1)*128],
                                 z1[:, o*GROUP:(o+1)*GROUP])
                # evac scatter: Z2 free = b*32 + o  (split DVE/ACT)
                dst = z2[:].rearrange("q (b o) -> q b o", o=32)[:, :, o]
                if o % 2 == 0:
                    nc.vector.tensor_copy(dst, ps_a[:])
                else:
                    nc.scalar.copy(dst, ps_a[:])

            # --- phase 3: 32x32 stream transpose -------------------------
            for s in range(16):
                sl = slice(s * 512, (s + 1) * 512)
                nc.vector.transpose(z3[:, sl], z2[:, sl])

            # --- phase 4: stage B matmuls (lhsT = data) ------------------
            z3v = z3[:].rearrange("p (b v) -> p b v", v=32)
            xo_tiles = [opool.tile([128, DIM], F32, tag="xo", name=f"xo_{g}_{c}")
                        for c in range(2)]
            for c in range(2):
                for vq in range(8):
                    ps_b = pb.tile([128, 512], F32, tag="pb")
                    for vv in range(4):
                        v = vq * 4 + vv
                        lhsT = z3v[:, c*128:(c+1)*128, v]    # [p, b] strided
                        nc.tensor.matmul(ps_b[:, vv*128:(vv+1)*128],
                                         lhsT,
                                         wb_sb[:, v*128:(v+1)*128])
                    # evac scatter: out free j' = o'*128 + j65*32 + v
                    dst = xo_tiles[c][:].rearrange(
                        "b (o f v) -> b v f o", f=4, v=32)[
                        :, vq*4:(vq+1)*4, :, :]
                    src = ps_b[:].rearrange("b (v f o) -> b v f o", v=4, f=4)
                    if vq % 4 == 3:
                        nc.vector.tensor_copy(dst, src)
                    else:
                        nc.scalar.copy(dst, src)

            # --- phase 5: store ------------------------------------------
            for c in range(2):
                row0 = g * GROUP + c * 128
                nc.sync.dma_start(out_d[row0:row0 + 128, :], xo_tiles[c][:])

    nc.compile()
    return nc


def _get_program():
    if "nc" not in _cache:
        _cache["nc"] = _build_program()
    return _cache["nc"]


# ---------------------------------------------------------------- entry
def kernel(x, angles):
    x = np.ascontiguousarray(np.asarray(x, dtype=np.float32))
    angles = np.asarray(angles, dtype=np.float32)
    assert x.shape == (BATCH, DIM) and angles.shape == (LAYERS, DIM // 2)

    WA, WB = _build_weights(angles)
    ident = np.eye(128, dtype=np.float32)
    nc = _get_program()

    in_maps = []
    for core in range(N_CORES):
        in_maps.append({
            "x": np.ascontiguousarray(x[core * BC:(core + 1) * BC]),
            "wa": WA, "wb": WB, "ident": ident,
        })
    res = bass_utils.run_bass_kernel_spmd(
        nc, in_maps, core_ids=list(range(N_CORES)))
    out = np.concatenate([r["out"] for r in res.results], axis=0)
    return out
